# revision 14
# baseline (speedup 1.0000x reference)
"""Trainium2 Bass kernel for a CGConv-style GNN message-passing layer.

kernel(**inputs) takes the FULL unsharded inputs (numpy) and returns
(atom_out [50000,64] f32, edge_out [800000,64] f32), matching the reference:

    msg  = sigmoid(z@Wf.T+bf) * softplus(z@Ws.T+bs) * exp(-d^2/18)
    agg  = segment_sum(msg, dst); atom_out = agg + atom_fea
    edge_out = silu(silu(z2@fc1.T+b1)@fc2.T+b2)

Sharding: edges are sorted by destination node; core c owns dst in
[c*6250,(c+1)*6250), so per-node aggregation is core-local (one-hot matmul
into PSUM per 128-node block) and only node tables are AllGathered.
Node projections are precomputed per node (P table) so the per-edge work is
two row gathers + one small matmul + elementwise ops.
"""
import os
import numpy as np
import ml_dtypes

import concourse.bass as bass
import concourse.tile as tile
from concourse import bacc, mybir
from concourse import bass_utils
from concourse import library_config

N_NODES = 50000
N_EDGES = 800000
F = 64
NCORES = 8
NPC = N_NODES // NCORES          # 6250 nodes per core
NBLK = (NPC + 127) // 128        # 49 blocks of 128 nodes
NPC_PAD = NBLK * 128             # 6272
HID = 14
HI_BASE = 4 * NPC_PAD            # padded-row base for the hi-half gathers
HI_THRESH = 32768                # prow >= this uses the hi gather base
BF = ml_dtypes.bfloat16

f32 = mybir.dt.float32
bf16 = mybir.dt.bfloat16
i16 = mybir.dt.int16
AF = mybir.ActivationFunctionType
ALU = mybir.AluOpType

_CACHE = {}


def _pack_idx(idx):
    """int16 index array -> dma_gather wrapped layout [128, n/16]."""
    n = idx.shape[0]
    return np.tile(idx.reshape(n // 16, 16).T, (8, 1)).copy()


def _prow(n):
    """Global node id -> row in the AllGathered padded table."""
    return (n // NPC) * NPC_PAD + (n % NPC)


def _build_structure(edge_idx):
    src = np.asarray(edge_idx[0], dtype=np.int64)
    dst = np.asarray(edge_idx[1], dtype=np.int64)
    E = src.shape[0]
    core = dst // NPC
    blk = (dst % NPC) // 128
    sprow = _prow(src)
    half = (sprow >= HI_THRESH).astype(np.int64)
    order = np.lexsort((half, blk, core))
    s_src, s_dst, s_core, s_blk, s_half = (
        src[order], dst[order], core[order], blk[order], half[order])

    counts = np.zeros((NCORES, NBLK, 2), dtype=np.int64)
    np.add.at(counts, (s_core, s_blk, s_half), 1)
    S_LO = int(np.ceil(counts[:, :, 0].max() / 128))
    S_HI = int(max(1, np.ceil(counts[:, :, 1].max() / 128)))
    S_BLK = S_LO + S_HI
    SLOTS = NBLK * S_BLK * 128

    base = np.cumsum(np.concatenate([[0], counts.reshape(-1)]))[:-1]
    base = base.reshape(NCORES, NBLK, 2)
    within = np.arange(E) - base[s_core, s_blk, s_half]
    slot = (s_blk * S_BLK + s_half * S_LO) * 128 + within
    return dict(order=order, slot=slot, s_src=s_src, s_dst=s_dst,
                s_core=s_core, S_LO=S_LO, S_HI=S_HI, S_BLK=S_BLK, SLOTS=SLOTS)


def _build_core_arrays(st, distance, edge_fea):
    S_LO, S_BLK, SLOTS = st["S_LO"], st["S_BLK"], st["SLOTS"]
    NS = NBLK * S_BLK
    cores = []
    for c in range(NCORES):
        m = st["s_core"] == c
        sl = st["slot"][m]
        esrc = st["s_src"][m]
        edst = st["s_dst"][m]
        eid = st["order"][m]

        ef = np.zeros((SLOTS, 2 * F), dtype=np.float32)
        ef[sl, 0:F] = edge_fea[eid]
        dist = np.full(SLOTS, 1.0e3, dtype=np.float32)
        dist[sl] = distance[eid]
        segid = np.zeros(SLOTS, dtype=np.float32)
        segid[sl] = (edst % NPC) % 128
        pi_idx = np.zeros(SLOTS, dtype=np.int16)
        pi_idx[sl] = (edst % NPC).astype(np.int16)
        sprow = _prow(esrc)
        pj_idx = np.zeros(SLOTS, dtype=np.int16)
        pj_idx[sl] = np.where(sprow >= HI_THRESH, sprow - HI_BASE, sprow).astype(np.int16)

        def lanes(a):
            return a.reshape(NS, 128).T.copy()

        cores.append(dict(
            ef_sorted=ef.astype(BF), dist=lanes(dist),
            segid=lanes(segid).astype(BF),
            pi_idx=_pack_idx(pi_idx), pj_idx=_pack_idx(pj_idx),
            slot=sl, eid=eid))
    return cores


def _build_bass(S_LO, S_HI):
    S_BLK = S_LO + S_HI
    NS = NBLK * S_BLK
    SLOTS = NS * 128
    nc = bacc.Bacc("TRN2", target_bir_lowering=False, debug=False,
                   num_devices=NCORES)

    def din(name, shape, dt):
        return nc.dram_tensor(name, shape, dt, kind="ExternalInput").ap()

    T = {}
    T["atom_local"] = din("atom_local", [NPC_PAD, F], f32)
    T["ef_sorted"] = din("ef_sorted", [SLOTS, 2 * F], bf16)
    T["dist_l"] = din("dist_l", [128, NS], f32)
    T["segid_l"] = din("segid_l", [128, NS], bf16)
    T["pi_idx"] = din("pi_idx", [128, SLOTS // 16], i16)
    T["pj_idx"] = din("pj_idx", [128, SLOTS // 16], i16)
    T["w_node"] = din("w_node", [F, 4 * F], f32)
    T["bias_row"] = din("bias_row", [1, 4 * F], f32)
    T["we_pack"] = din("we_pack", [F, 2 * F], bf16)
    T["w1ab"] = din("w1ab", [F, 2 * HID], f32)
    T["w1e"] = din("w1e", [F, HID], bf16)
    T["fc2wT"] = din("fc2wT", [HID, F], bf16)
    T["fc1b_col"] = din("fc1b_col", [HID, 1], f32)
    T["fc2b_col"] = din("fc2b_col", [F, 1], f32)
    T["ident_bf"] = din("ident_bf", [128, 128], bf16)
    T["ident_f32"] = din("ident_f32", [128, 128], f32)
    T["iota_row"] = din("iota_row", [128, 128], bf16)
    T["ones_col"] = din("ones_col", [1, 128], f32)

    T["atom_out_sl"] = nc.dram_tensor("atom_out_sl", [NPC_PAD, F], f32,
                                      kind="ExternalOutput").ap()
    T["eout_T"] = nc.dram_tensor("eout_T", [F, SLOTS], f32,
                                 kind="ExternalOutput").ap()

    T["p_slice"] = nc.dram_tensor("p_slice", [NPC_PAD, 4 * F], bf16)
    T["p_all"] = nc.dram_tensor("p_all", [NPC_PAD * NCORES, 4 * F], bf16,
                                addr_space="Shared")
    T["p_all_l"] = nc.dram_tensor("p_all_l", [NPC_PAD * NCORES, 4 * F], bf16)
    T["u_slice"] = nc.dram_tensor("u_slice", [NPC_PAD, F], f32)
    T["u_all"] = nc.dram_tensor("u_all", [NPC_PAD * NCORES, F], f32,
                                addr_space="Shared")
    T["u_all_l"] = nc.dram_tensor("u_all_l", [NPC_PAD * NCORES, F], f32)

    with tile.TileContext(nc) as tc:
        _kernel_body(nc, tc, T, S_LO, S_HI)
    nc.compile()
    return nc


def _kernel_body(nc, tc, T, S_LO, S_HI):
    S_BLK = S_LO + S_HI
    NS = NBLK * S_BLK
    SLOTS = NS * 128
    from contextlib import ExitStack
    ctx = ExitStack()
    with ctx:
        const = ctx.enter_context(tc.tile_pool(name="const", bufs=1))
        big = ctx.enter_context(tc.tile_pool(name="big", bufs=1))
        sb = ctx.enter_context(tc.tile_pool(name="sb", bufs=2))
        sb3 = ctx.enter_context(tc.tile_pool(name="sb3", bufs=2))
        gat = ctx.enter_context(tc.tile_pool(name="gat", bufs=2))
        wideps = ctx.enter_context(tc.tile_pool(name="wideps", bufs=1, space="PSUM"))
        aggps = ctx.enter_context(tc.tile_pool(name="aggps", bufs=2, space="PSUM"))
        miscps = ctx.enter_context(tc.tile_pool(name="miscps", bufs=1, space="PSUM"))

        nc.gpsimd.load_library(library_config.mlp)

        def load_const(ap, shape, dt, tag):
            t = const.tile(shape, dt, tag=tag)
            nc.sync.dma_start(t[:], ap[:, :])
            return t

        w_node_t = load_const(T["w_node"], [F, 4 * F], f32, "w_node")
        bias_row_t = load_const(T["bias_row"], [1, 4 * F], f32, "bias_row")
        we_t = load_const(T["we_pack"], [F, 2 * F], bf16, "we")
        w1ab_t = load_const(T["w1ab"], [F, 2 * HID], f32, "w1ab")
        w1e_t = load_const(T["w1e"], [F, HID], bf16, "w1e")
        fc2wT_t = load_const(T["fc2wT"], [HID, F], bf16, "fc2wT")
        fc1b_t = load_const(T["fc1b_col"], [HID, 1], f32, "fc1b")
        fc2b_t = load_const(T["fc2b_col"], [F, 1], f32, "fc2b")
        ibf_t = load_const(T["ident_bf"], [128, 128], bf16, "ibf")
        if32_t = load_const(T["ident_f32"], [128, 128], f32, "if32")
        iota_t = load_const(T["iota_row"], [128, 128], bf16, "iota")
        ones_t = load_const(T["ones_col"], [1, 128], f32, "ones")

        idx_pi = big.tile([128, SLOTS // 16], i16, tag="idx_pi")
        nc.sync.dma_start(idx_pi[:], T["pi_idx"][:, :])
        idx_pj = big.tile([128, SLOTS // 16], i16, tag="idx_pj")
        nc.sync.dma_start(idx_pj[:], T["pj_idx"][:, :])
        dist_t = big.tile([128, NS], f32, tag="dist")
        nc.sync.dma_start(dist_t[:], T["dist_l"][:, :])
        segid_t = big.tile([128, NS], bf16, tag="segid")
        nc.sync.dma_start(segid_t[:], T["segid_l"][:, :])

        g_t = big.tile([128, NS], f32, tag="g")
        nc.scalar.activation(g_t[:], dist_t[:], AF.Square)
        nc.scalar.activation(g_t[:], g_t[:], AF.Exp, scale=-1.0 / 18.0)

        # ---------------- P-table build (own node slice) ----------------
        atom_sb = []
        for b in range(NBLK):
            at = const.tile([128, F], f32, tag=f"atomblk{b}")
            nc.sync.dma_start(at[:], T["atom_local"][b * 128:(b + 1) * 128, :])
            atom_sb.append(at)
            atT_ps = miscps.tile([F, 128], f32, tag="tps")
            nc.tensor.transpose(atT_ps[:], at[:], if32_t[:])
            atT = sb.tile([F, 128], f32, tag="tps_sb")
            nc.vector.tensor_copy(atT[:], atT_ps[:])
            pp = miscps.tile([128, 4 * F], f32, tag="mm_small")
            nc.tensor.matmul(pp[:], atT[:], w_node_t[:], start=True, stop=False)
            nc.tensor.matmul(pp[:], ones_t[:], bias_row_t[:], start=False, stop=True)
            pbf = sb.tile([128, 4 * F], bf16, tag="pbuild_out")
            nc.vector.tensor_copy(pbf[:], pp[:])
            nc.sync.dma_start(T["p_slice"][b * 128:(b + 1) * 128, :], pbf[:])

        nc.gpsimd.collective_compute(
            "AllGather", ALU.bypass, replica_groups=[list(range(NCORES))],
            ins=[T["p_slice"].ap().opt()], outs=[T["p_all"].ap().opt()])

        nc.sync.dma_start(T["p_all_l"].ap(), T["p_all"].ap())

        if int(os.environ.get("K_PHASE", "3")) < 1:
            return

        # ---------------- Phase 1: messages + scatter -------------------
        p_all_ap = T["p_all_l"].ap()
        p_loc_ap = T["p_slice"].ap()

        nblk1 = min(NBLK, int(os.environ.get("K_BLOCKS", str(NBLK))))
        for b in range(nblk1):
            agg = aggps.tile([128, F], f32, tag="agg")
            for half, (w0, W) in enumerate([(0, S_LO), (S_LO, S_HI)]):
                st0 = b * S_BLK + w0          # first subtile of window
                c0, c1 = st0 * 128, (st0 + W) * 128

                pi_g = gat.tile([128, W, 2 * F], bf16, tag="pi")
                pj_g = gat.tile([128, W, 2 * F], bf16, tag="pj")
                src_ap = p_all_ap if half == 0 else p_all_ap[HI_BASE:, :]
                for q0 in range(0, W, 4):
                    q1 = min(q0 + 4, W)
                    nq = (q1 - q0) * 128
                    nc.gpsimd.dma_gather(
                        out_ap=pi_g[:, q0:q1, :], in_ap=p_loc_ap[:, 0:2 * F],
                        idxs_ap=idx_pi[:, (st0 + q0) * 8:(st0 + q1) * 8],
                        num_idxs=nq, num_idxs_reg=nq,
                        elem_size=2 * F, elem_step=4 * F)
                    nc.gpsimd.dma_gather(
                        out_ap=pj_g[:, q0:q1, :], in_ap=src_ap[:, 2 * F:4 * F],
                        idxs_ap=idx_pj[:, (st0 + q0) * 8:(st0 + q1) * 8],
                        num_idxs=nq, num_idxs_reg=nq,
                        elem_size=2 * F, elem_step=4 * F)

                efT = sb3.tile([128, W * 128], bf16, tag="efT")
                if int(os.environ.get("K_EFT", "1")):
                    nc.sync.dma_start(efT[:], T["ef_sorted"][c0:c1, :], transpose=True)
                else:
                    nc.gpsimd.memset(efT[:], 0.0)

                zps = wideps.tile([128, W, 128], f32, tag="wide")
                for s in range(W):
                    nc.tensor.matmul(zps[:, s, :],
                                     efT[0:F, s * 128:(s + 1) * 128],
                                     we_t[:], start=True, stop=False)
                    nc.tensor.matmul(zps[:, s, :], ibf_t[:], pi_g[:, s, :],
                                     start=False, stop=False)
                    nc.tensor.matmul(zps[:, s, :], ibf_t[:], pj_g[:, s, :],
                                     start=False, stop=True)

                ea = sb.tile([128, W, 128], f32, tag="ea")
                nc.scalar.activation(ea[:], zps[:], AF.Exp)
                sp = sb.tile([128, W, 128], f32, tag="sp")
                nc.scalar.activation(sp[:], ea[:], AF.Ln, bias=1.0)
                uu = sb.tile([128, W, F], f32, tag="u")
                nc.vector.tensor_tensor(uu[:], zps[:, :, 0:F], sp[:, :, 0:F],
                                        ALU.subtract)
                sg = sb.tile([128, W, F], f32, tag="sg")
                nc.scalar.activation(sg[:], uu[:], AF.Exp)
                pr = sb.tile([128, W, F], f32, tag="pr")
                nc.vector.tensor_tensor(pr[:], sg[:], sp[:, :, F:2 * F], ALU.mult)
                msg = sb.tile([128, W, F], bf16, tag="msg")
                gsl = g_t[:, st0:st0 + W].unsqueeze(2).broadcast_to([128, W, F])
                nc.vector.tensor_tensor(msg[:], pr[:], gsl, ALU.mult)

                oh = sb.tile([128, W, 128], bf16, tag="oh")
                ssl = segid_t[:, st0:st0 + W].unsqueeze(2).broadcast_to([128, W, 128])
                isl = iota_t[:].unsqueeze(1).broadcast_to([128, W, 128])
                nc.vector.tensor_tensor(oh[:], ssl, isl, ALU.is_equal)

                for s in range(W):
                    nc.tensor.matmul(agg[:], oh[:, s, :], msg[:, s, :],
                                     start=(half == 0 and s == 0),
                                     stop=(half == 1 and s == W - 1))

            ao = sb.tile([128, F], f32, tag="ao")
            nc.vector.tensor_tensor(ao[:], agg[:], atom_sb[b][:], ALU.add)
            nc.sync.dma_start(T["atom_out_sl"][b * 128:(b + 1) * 128, :], ao[:])

            aoT_ps = miscps.tile([F, 128], f32, tag="tps")
            nc.tensor.transpose(aoT_ps[:], ao[:], if32_t[:])
            aoT = sb.tile([F, 128], f32, tag="tps_sb")
            nc.vector.tensor_copy(aoT[:], aoT_ps[:])
            up = miscps.tile([128, 2 * HID], f32, tag="mm_small")
            nc.tensor.matmul(up[:], aoT[:], w1ab_t[:], start=True, stop=True)
            uo = sb.tile([128, F], f32, tag="uo")
            nc.gpsimd.memset(uo[:, 2 * HID:F], 0.0)
            nc.vector.tensor_copy(uo[:, 0:2 * HID], up[:])
            nc.sync.dma_start(T["u_slice"][b * 128:(b + 1) * 128, :], uo[:])

        nc.gpsimd.collective_compute(
            "AllGather", ALU.bypass, replica_groups=[list(range(NCORES))],
            ins=[T["u_slice"].ap().opt()], outs=[T["u_all"].ap().opt()])

        if int(os.environ.get("K_PHASE", "3")) < 3:
            return

        nc.sync.dma_start(T["u_all_l"].ap(), T["u_all"].ap())

        # ---------------- Phase 2: edge-update MLP ----------------------
        u_all_ap = T["u_all_l"].ap()
        u_loc_ap = T["u_slice"].ap()
        for b in range(nblk1):
            for half, (w0, W) in enumerate([(0, S_LO), (S_LO, S_HI)]):
                st0 = b * S_BLK + w0
                c0, c1 = st0 * 128, (st0 + W) * 128

                src_ap = u_all_ap if half == 0 else u_all_ap[HI_BASE:, :]
                ua_g = gat.tile([128, W, F], f32, tag="ua")
                ub_g = gat.tile([128, W, F], f32, tag="ub")
                for q0 in range(0, W, 4):
                    q1 = min(q0 + 4, W)
                    nq = (q1 - q0) * 128
                    nc.gpsimd.dma_gather(
                        out_ap=ua_g[:, q0:q1, :], in_ap=src_ap[:, :],
                        idxs_ap=idx_pj[:, (st0 + q0) * 8:(st0 + q1) * 8],
                        num_idxs=nq, num_idxs_reg=nq,
                        elem_size=F, elem_step=F)
                    nc.gpsimd.dma_gather(
                        out_ap=ub_g[:, q0:q1, :], in_ap=u_loc_ap[:, :],
                        idxs_ap=idx_pi[:, (st0 + q0) * 8:(st0 + q1) * 8],
                        num_idxs=nq, num_idxs_reg=nq,
                        elem_size=F, elem_step=F)
                uab = sb.tile([128, W, HID], f32, tag="uab")
                nc.vector.tensor_tensor(uab[:], ua_g[:, :, 0:HID],
                                        ub_g[:, :, HID:2 * HID], ALU.add)

                efT = sb3.tile([128, W * 128], bf16, tag="efT")
                nc.sync.dma_start(efT[:], T["ef_sorted"][c0:c1, :], transpose=True)

                hp = wideps.tile([HID, W * 128], f32, tag="wide")
                for q0 in range(0, W, 4):
                    q1 = min(q0 + 4, W)
                    nc.tensor.matmul(hp[:, q0 * 128:q1 * 128], w1e_t[:],
                                     efT[0:F, q0 * 128:q1 * 128],
                                     start=True, stop=False)
                    for s in range(q0, q1):
                        nc.tensor.matmul(
                            hp[:, s * 128:(s + 1) * 128], uab[:, s, :],
                            if32_t[:], is_transpose=True,
                            start=False, stop=(s == q1 - 1))
                t1 = sb.tile([HID, W * 128], f32, tag="t1")
                nc.scalar.activation(t1[:], hp[:], AF.Identity, bias=fc1b_t[:, 0:1])
                s1 = sb.tile([HID, W * 128], f32, tag="s1")
                nc.scalar.activation(s1[:], t1[:], AF.Sigmoid)
                hs = sb.tile([HID, W * 128], bf16, tag="hs")
                nc.vector.tensor_tensor(hs[:], t1[:], s1[:], ALU.mult)
                ep = wideps.tile([F, W * 128], f32, tag="wide")
                for q0 in range(0, W, 4):
                    q1 = min(q0 + 4, W)
                    nc.tensor.matmul(ep[:, q0 * 128:q1 * 128], fc2wT_t[:],
                                     hs[:, q0 * 128:q1 * 128],
                                     start=True, stop=True)
                t2 = sb.tile([F, W * 128], f32, tag="t2")
                nc.scalar.activation(t2[:], ep[:], AF.Identity, bias=fc2b_t[:, 0:1])
                s2 = sb.tile([F, W * 128], f32, tag="s2")
                nc.scalar.activation(s2[:], t2[:], AF.Sigmoid)
                eo = sb.tile([F, W * 128], f32, tag="eo")
                nc.vector.tensor_tensor(eo[:], t2[:], s2[:], ALU.mult)
                nc.sync.dma_start(T["eout_T"][:, c0:c1], eo[:])


def kernel(**inputs):
    atom_fea = np.asarray(inputs["atom_fea"], dtype=np.float32)
    edge_idx = np.asarray(inputs["edge_idx"])
    edge_fea = np.asarray(inputs["edge_fea"], dtype=np.float32)
    distance = np.asarray(inputs["distance"], dtype=np.float32)
    lin_f_w = np.asarray(inputs["lin_f_w"], dtype=np.float32)
    lin_f_b = np.asarray(inputs["lin_f_b"], dtype=np.float32)
    lin_s_w = np.asarray(inputs["lin_s_w"], dtype=np.float32)
    lin_s_b = np.asarray(inputs["lin_s_b"], dtype=np.float32)
    fc1_w = np.asarray(inputs["fc1_w"], dtype=np.float32)
    fc1_b = np.asarray(inputs["fc1_b"], dtype=np.float32)
    fc2_w = np.asarray(inputs["fc2_w"], dtype=np.float32)
    fc2_b = np.asarray(inputs["fc2_b"], dtype=np.float32)

    st = _build_structure(edge_idx)
    cores = _build_core_arrays(st, distance, edge_fea)
    S_LO, S_HI = st["S_LO"], st["S_HI"]

    key = (S_LO, S_HI)
    if key not in _CACHE:
        _CACHE[key] = _build_bass(S_LO, S_HI)
    nc = _CACHE[key]

    Wfi, Wfj, Wfe = lin_f_w[:, 0:F], lin_f_w[:, F:2 * F], lin_f_w[:, 2 * F:]
    Wsi, Wsj, Wse = lin_s_w[:, 0:F], lin_s_w[:, F:2 * F], lin_s_w[:, 2 * F:]
    w_node = np.concatenate([Wfi.T, Wsi.T, Wfj.T, Wsj.T], axis=1).astype(np.float32)
    bias_row = np.concatenate([lin_f_b, lin_s_b, np.zeros(2 * F, np.float32)])[None, :]
    we_pack = np.concatenate([Wfe.T, Wse.T], axis=1).astype(BF)
    W1a, W1b, W1e = fc1_w[:, 0:F], fc1_w[:, F:2 * F], fc1_w[:, 2 * F:]
    w1ab = np.concatenate([W1a.T, W1b.T], axis=1).astype(np.float32)

    ident = np.eye(128, dtype=np.float32)
    iota_row = np.tile(np.arange(128, dtype=np.float32)[None, :], (128, 1))
    atom_pad = np.zeros((NPC_PAD, F), dtype=np.float32)

    common = dict(
        w_node=w_node, bias_row=bias_row.astype(np.float32),
        we_pack=we_pack, w1ab=w1ab, w1e=W1e.T.astype(BF),
        fc2wT=fc2_w.T.astype(BF),
        fc1b_col=np.ascontiguousarray(fc1_b[:, None], dtype=np.float32),
        fc2b_col=np.ascontiguousarray(fc2_b[:, None], dtype=np.float32),
        ident_bf=ident.astype(BF), ident_f32=ident,
        iota_row=iota_row.astype(BF),
        ones_col=np.ones((1, 128), dtype=np.float32))

    in_maps = []
    for c in range(NCORES):
        ap = atom_pad.copy()
        ap[0:NPC] = atom_fea[c * NPC:(c + 1) * NPC]
        in_maps.append(dict(
            atom_local=ap, ef_sorted=cores[c]["ef_sorted"],
            dist_l=cores[c]["dist"], segid_l=cores[c]["segid"],
            pi_idx=cores[c]["pi_idx"], pj_idx=cores[c]["pj_idx"],
            **common))

    res = bass_utils.run_bass_kernel_spmd(nc, in_maps, core_ids=list(range(NCORES)))
    kernel.last_results = res

    atom_out = np.empty((N_NODES, F), dtype=np.float32)
    edge_out = np.empty((N_EDGES, F), dtype=np.float32)
    for c in range(NCORES):
        atom_out[c * NPC:(c + 1) * NPC] = res.results[c]["atom_out_sl"][0:NPC]
        eo = res.results[c]["eout_T"]
        edge_out[cores[c]["eid"]] = eo[:, cores[c]["slot"]].T
    return atom_out, edge_out
